# revision 22
# baseline (speedup 1.0000x reference)
"""Trainium2 Bass kernel for nn_Attention_16011638079620 (gnn_message_passing).

Computes, for feats [8192, 256] f32 and kn=10:
    sim   = cosine-similarity(feats)            [N, N]
    B     = rowwise top-kn one-hot mask of softmax(sim) (rank-preserving)
    G     = (1/kn) * invdv_i * invdv_j * (B^T B)_ij,  dv = colsums of B

Strategy (8 cores):
  - sim via 3-pass bf16 hi/lo split matmuls (exact top-k, 4x faster than f32)
  - B columns packed 3-per-fp16 value (base 24; counts <= 23 so the packed
    matmul B^T @ packedB is integer-exact in fp32 PSUM) -> 1.5x fp8-DR rate
  - G is symmetric: core c computes blocks (c, (c+d)%8) for d=0..4; host
    mirrors the rest. lhsT is always the core's own column slice (AllToAll).
  - dv via fp8-DoubleRow ones-matmuls, interleaved with the sim phase.
  - software pipeline: mask/pack of block m-1 runs behind sim of block m so
    PSUM banks release early; collectives merged in block pairs.
"""

import sys

sys.path.insert(0, "/opt/trn_rl_repo")

from contextlib import ExitStack

import numpy as np

import concourse.bass as bass
import concourse.tile as tile
from concourse import bacc, mybir
from concourse.bass import _add_dep_helper
from concourse.bass_utils import run_bass_kernel_spmd

f32 = mybir.dt.float32
bf16 = mybir.dt.bfloat16
fp16 = mybir.dt.float16
fp8 = mybir.dt.float8e4
i32 = mybir.dt.int32
Alu = mybir.AluOpType
Act = mybir.ActivationFunctionType
NEG = -1e30
BASE = 24.0
B2 = BASE * BASE  # 576
MAGIC = 12582912.0  # 1.5 * 2**23: (z + MAGIC) - MAGIC == round-to-nearest(z)
CB2 = 0.5 / B2 - 0.5  # bias so round(P/576 + CB2) == floor(P/576) exactly
CB1 = 0.5 / BASE - 0.5


def build_nc(N, D, KN, NCORES):
    RP = N // NCORES           # 1024 rows/G-rows per core
    MB = RP // 128             # 8 row blocks per core
    NCH = N // 512             # topk chunks
    DT = D // 128              # 2 feature chunks
    PC = 342                   # packed cols per 1024-col slice (342*3 = 1026)
    PCW = NCORES * PC          # 2736 packed cols total per row
    PAD = 3 * PC               # 1026 padded cols per slice
    BW = NCORES * PAD          # 8208 padded mask width
    KK = N // 128              # 64 contraction chunks for phase D
    NP = MB // 2               # 4 block pairs for collectives
    ND = 5                     # symmetric blocks per core
    assert 8 < KN <= 16

    inv_de = float(np.float32(1.0) / np.float32(KN))

    nc = bacc.Bacc(
        "TRN2",
        target_bir_lowering=False,
        debug=False,
        enable_asserts=False,
        num_devices=NCORES,
    )
    feats_all = nc.dram_tensor("feats_all", [N, D], f32, kind="ExternalInput").ap()
    feats_my = nc.dram_tensor("feats_my", [RP, D], f32, kind="ExternalInput").ap()
    ident_in = nc.dram_tensor("ident_in", [128, 128], f32, kind="ExternalInput").ap()
    jsel_in = nc.dram_tensor("jsel_in", [1, 8], i32, kind="ExternalInput").ap()
    g_out = nc.dram_tensor("g_out", [RP, ND * RP], f32, kind="ExternalOutput").ap()

    rg = [list(range(NCORES))]

    with tile.TileContext(nc) as tc, ExitStack() as ctx:
        dram = ctx.enter_context(tc.tile_pool(name="dram", bufs=1, space="DRAM"))
        b_grp2 = [
            dram.tile([NCORES, 2, 128, RP], fp8, name=f"b_grp2_{p}") for p in range(NP)
        ]
        lts_d2 = [
            dram.tile([NCORES, 2, 128, RP], fp8, name=f"lts_d2_{p}") for p in range(NP)
        ]
        pk_in2 = [dram.tile([2, 128, PCW], fp16, name=f"pk_in2_{p}") for p in range(NP)]
        pk_ag2 = [
            dram.tile(
                [NCORES, 2, 128, PCW], fp16, addr_space="Shared", name=f"pk_ag2_{p}"
            )
            for p in range(NP)
        ]
        dv_my_d = dram.tile([RP], f32, name="dv_my_d")
        dv_full = dram.tile([N], f32, addr_space="Shared", name="dv_full")
        cs_dram = dram.tile([N], f32, name="cs_dram")

        pers = ctx.enter_context(tc.tile_pool(name="pers", bufs=1))
        dv_stack = ExitStack()
        dvps_pool = dv_stack.enter_context(
            tc.tile_pool(name="dvps", bufs=1, space="PSUM")
        )
        dv_ps = dvps_pool.tile([128, MB], f32, name="dv_ps")

        ident = pers.tile([128, 128], f32, name="ident")
        nc.sync.dma_start(ident[:], ident_in)
        idb = pers.tile([128, 128], bf16, name="idb")
        nc.vector.tensor_copy(idb[:], ident[:])
        jsel_sb = pers.tile([1, 8], i32, name="jsel_sb")
        nc.sync.dma_start(jsel_sb[:], jsel_in)
        ones1 = pers.tile([1, 128], f32, name="ones1")
        nc.vector.memset(ones1[:], 1.0)
        ones_dr = pers.tile([128, 2, 16], fp8, name="ones_dr")
        nc.vector.memset(ones_dr[:], 1.0)

        # lhsT for phase D: [128, slot = m'*NCORES+o, my 1024 cols] fp8 (8MB)
        lt_all = pers.tile([128, KK, RP], fp8, name="lt_all")

        # ---------------- phase 1: normalize + hi/lo split + transpose ------
        with ExitStack() as p12:
            fsb = p12.enter_context(tc.tile_pool(name="fsb", bufs=1))
            fnt_hi = [fsb.tile([128, N], bf16, name=f"fh{h}") for h in range(DT)]
            fnt_lo = [fsb.tile([128, N], bf16, name=f"fl{h}") for h in range(DT)]
            fnt_myh = [fsb.tile([128, RP], bf16, name=f"fmh{h}") for h in range(DT)]
            fnt_myl = [fsb.tile([128, RP], bf16, name=f"fml{h}") for h in range(DT)]

            with ExitStack() as p1:
                wrk = p1.enter_context(tc.tile_pool(name="wrk", bufs=3))
                sml = p1.enter_context(tc.tile_pool(name="sml", bufs=6))
                tp_ps = p1.enter_context(
                    tc.tile_pool(name="tp_ps", bufs=1, space="PSUM")
                )

                def norm_group(src4, dh, dl, col0, nb):
                    # nb row-blocks batched: one op set for the whole group
                    ft4 = wrk.tile([128, nb, D], f32, name="ft4")
                    nc.sync.dma_start(ft4[:], src4)
                    tps = {}
                    for x in range(2):
                        for h in range(DT):
                            tps[(x, h)] = tp_ps.tile(
                                [128, nb * 128], bf16, name=f"tp{x}{h}", tag=f"tp{x}{h}"
                            )
                    sq4 = wrk.tile([128, nb, D], f32, name="sq4")
                    nc.scalar.square(
                        sq4.rearrange("p b d -> p (b d)"),
                        ft4.rearrange("p b d -> p (b d)"),
                    )
                    n24 = sml.tile([128, nb, 1], f32, name="n24")
                    nc.vector.reduce_sum(n24[:], sq4[:], axis=mybir.AxisListType.X)
                    nrm4 = sml.tile([128, nb, 1], f32, name="nrm4")
                    nc.scalar.sqrt(
                        nrm4.rearrange("p b o -> p (b o)"),
                        n24.rearrange("p b o -> p (b o)"),
                    )
                    inv4 = sml.tile([128, nb, 1], f32, name="inv4")
                    nc.vector.reciprocal(
                        inv4.rearrange("p b o -> p (b o)"),
                        nrm4.rearrange("p b o -> p (b o)"),
                    )
                    fn4 = wrk.tile([128, nb, D], f32, name="fn4")
                    nc.vector.tensor_tensor(
                        fn4[:], ft4[:], inv4[:].broadcast_to([128, nb, D]),
                        op=Alu.mult,
                    )
                    fh4 = wrk.tile([128, nb, D], bf16, name="fh4")
                    nc.scalar.copy(
                        fh4.rearrange("p b d -> p (b d)"),
                        fn4.rearrange("p b d -> p (b d)"),
                    )
                    fl4 = wrk.tile([128, nb, D], bf16, name="fl4")
                    nc.vector.tensor_tensor(
                        fl4[:], fn4[:], fh4[:], op=Alu.subtract
                    )
                    for i in range(nb):
                        for h in range(DT):
                            for x, src in ((0, fh4), (1, fl4)):
                                nc.tensor.transpose(
                                    tps[(x, h)][:, i * 128 : (i + 1) * 128],
                                    src[:, i, h * 128 : (h + 1) * 128],
                                    idb[:],
                                )
                    for h in range(DT):
                        for x, dst in ((0, dh), (1, dl)):
                            nc.scalar.copy(
                                dst[h][:, col0 : col0 + nb * 128], tps[(x, h)][:]
                            )

                fm4 = feats_my.rearrange("(g i p) d -> g p i d", p=128, i=4)
                for g in range(MB // 4):
                    norm_group(fm4[g], fnt_myh, fnt_myl, g * 512, 4)
                fa4 = feats_all.rearrange("(g i p) d -> g p i d", p=128, i=4)
                for g in range(N // 512):
                    norm_group(fa4[g], fnt_hi, fnt_lo, g * 512, 4)

            # ---------------- phase 2: sim, topk, mask, pack, CC -----------
            with ExitStack() as p2:
                simp = p2.enter_context(tc.tile_pool(name="simp", bufs=3))
                smal = p2.enter_context(tc.tile_pool(name="smal", bufs=2))
                bmpp = p2.enter_context(tc.tile_pool(name="bmpp", bufs=1))
                pkp = p2.enter_context(tc.tile_pool(name="pkp", bufs=1))
                t0p = p2.enter_context(tc.tile_pool(name="t0p", bufs=2))
                sim_ps = p2.enter_context(
                    tc.tile_pool(name="sim_ps", bufs=1, space="PSUM")
                )
                combos = []
                for h in range(DT):
                    combos.append((fnt_myh[h], fnt_hi[h]))
                    combos.append((fnt_myh[h], fnt_lo[h]))
                    combos.append((fnt_myl[h], fnt_hi[h]))

                tkns = {}
                halves = {}

                def sim_block(m):
                    sh0 = simp.tile([128, N // 2], f32, name="sh0", tag="sh")
                    sh1 = simp.tile([128, N // 2], f32, name="sh1", tag="sh")
                    halves[m] = (sh0, sh1)
                    cand = smal.tile([128, 8 * NCH], f32, name="cand", tag="cand")
                    for qr in range(4):
                        pss = [
                            sim_ps.tile([128, 512], f32, name=f"sq{t}", tag=f"sq{t}")
                            for t in range(4)
                        ]
                        for ci, (la, ra) in enumerate(combos):
                            lt = la[:, m * 128 : (m + 1) * 128]
                            for t in range(4):
                                ntc = qr * 4 + t
                                nc.tensor.matmul(
                                    pss[t][:],
                                    lt,
                                    ra[:, ntc * 512 : (ntc + 1) * 512],
                                    start=(ci == 0),
                                    stop=(ci == 5),
                                )
                        sh = (sh0, sh1)[qr // 2]
                        for t in range(4):
                            ntc = qr * 4 + t
                            nc.vector.max(
                                cand[:, ntc * 8 : (ntc + 1) * 8], pss[t][:]
                            )
                            nc.scalar.copy(
                                sh[:, (ntc % 8) * 512 : (ntc % 8 + 1) * 512],
                                pss[t][:],
                            )
                    c8 = smal.tile([128, 8], f32, name="c8", tag="c8")
                    nc.vector.max(c8[:], cand[:])
                    cand2 = smal.tile([128, 8 * NCH], f32, name="cand2", tag="cand2")
                    nc.vector.match_replace(cand2[:], c8[:], cand[:], NEG)
                    c8b = smal.tile([128, 8], f32, name="c8b", tag="c8b")
                    nc.vector.max(c8b[:], cand2[:])
                    tkns[m] = c8b

                def mask_pack(m):
                    tkn = tkns[m][:, KN - 9 : KN - 8]
                    sh0, sh1 = halves[m]
                    bmp = bmpp.tile([128, BW], fp8, name="bmp")
                    for j in range(NCORES):
                        sh = (sh0, sh1)[j // 4]
                        nc.vector.tensor_scalar(
                            bmp[:, j * PAD : j * PAD + RP],
                            sh[:, (j % 4) * RP : (j % 4 + 1) * RP],
                            tkn,
                            None,
                            op0=Alu.is_ge,
                        )
                        nc.vector.memset(bmp[:, j * PAD + RP : (j + 1) * PAD], 0.0)
                    pk = pkp.tile([128, PCW], fp16, name="pk")
                    bm3 = bmp.rearrange("p (j t u) -> p j t u", j=NCORES, u=3)
                    for j in range(NCORES):
                        t0 = t0p.tile([128, PC], f32, name="t0")
                        nc.vector.scalar_tensor_tensor(
                            t0[:], in0=bm3[:, j, :, 1], scalar=BASE,
                            in1=bm3[:, j, :, 0], op0=Alu.mult, op1=Alu.add,
                        )
                        nc.vector.scalar_tensor_tensor(
                            pk[:, j * PC : (j + 1) * PC],
                            in0=bm3[:, j, :, 2], scalar=B2, in1=t0[:],
                            op0=Alu.mult, op1=Alu.add,
                        )
                    pp, i = m // 2, m % 2
                    nc.sync.dma_start(pk_in2[pp][i], pk[:])
                    nc.sync.dma_start(
                        b_grp2[pp][:, i].rearrange("j p q -> p j q"),
                        bmp.rearrange("p (j q) -> p j q", j=NCORES)[:, :, 0:RP],
                    )

                lt_dma = {}

                def cc_pair(pp):
                    nc.gpsimd.collective_compute(
                        "AllToAll", Alu.bypass, replica_groups=rg,
                        ins=[b_grp2[pp].opt()], outs=[lts_d2[pp].opt()],
                    )
                    nc.gpsimd.collective_compute(
                        "AllGather", Alu.bypass, replica_groups=rg,
                        ins=[pk_in2[pp].opt()], outs=[pk_ag2[pp].opt()],
                    )
                    for i in range(2):
                        mp = 2 * pp + i
                        lt_dma[mp] = nc.sync.dma_start(
                            lt_all[:, mp * NCORES : (mp + 1) * NCORES, :],
                            lts_d2[pp][:, i].rearrange("o p q -> p o q"),
                        )

                ltp = lt_all.rearrange("p (t i o) q -> p t i o q", i=2, o=NCORES)

                def dv_pair(pp):
                    first = True
                    for o in range(NCORES):
                        for m in range(MB):
                            mm = nc.tensor.matmul(
                                dv_ps[:, m : m + 1],
                                ltp[:, pp, :, o, m * 128 : (m + 1) * 128],
                                ones_dr[:, :, 0:1],
                                perf_mode=mybir.MatmulPerfMode.DoubleRow,
                                start=(pp == 0 and o == 0 and m == 0),
                                stop=(pp == NP - 1 and o == NCORES - 1 and m == MB - 1),
                                skip_group_check=True,
                            )
                            if first:
                                for mp in (2 * pp, 2 * pp + 1):
                                    _add_dep_helper(
                                        mm.ins, lt_dma[mp].ins, sync=True,
                                        reason="dv after lt load",
                                    )
                                first = False

                for m in range(MB):
                    sim_block(m)
                    if m >= 1:
                        mask_pack(m - 1)
                        if (m - 1) % 2 == 1:
                            cc_pair((m - 1) // 2)
                    if m >= 4 and m % 2 == 0:
                        dv_pair((m - 4) // 2)
                mask_pack(MB - 1)
                cc_pair(NP - 1)
                dv_pair(NP - 2)
                dv_pair(NP - 1)
                lt_dmas = lt_dma

        # ---------------- phase 3+4: dv scales, G blocks --------------------
        with ExitStack() as p4:
            gw = p4.enter_context(tc.tile_pool(name="gw", bufs=4))
            dv_sb = gw.tile([128, MB], f32, name="dv_sb")
            nc.vector.tensor_copy(dv_sb[:], dv_ps[:])
            dv_stack.close()
            csp = p4.enter_context(tc.tile_pool(name="csp", bufs=2))
            stg = p4.enter_context(tc.tile_pool(name="stg", bufs=2))
            esp = p4.enter_context(tc.tile_pool(name="esp", bufs=2))
            rhp = p4.enter_context(tc.tile_pool(name="rhp", bufs=MB + 1))
            g_ps = p4.enter_context(tc.tile_pool(name="g_ps", bufs=1, space="PSUM"))
            nc.sync.dma_start(dv_my_d.rearrange("(m p) -> p m", p=128), dv_sb[:])
            nc.gpsimd.collective_compute(
                "AllGather", Alu.bypass, replica_groups=rg,
                ins=[dv_my_d.opt()], outs=[dv_full.opt()],
            )
            # rowscale: rs = invdv(my col block) * inv_de
            d1 = gw.tile([128, MB], f32, name="d1")
            nc.vector.tensor_scalar_max(d1[:], dv_sb[:], 1.0)
            sq = gw.tile([128, MB], f32, name="sqv")
            nc.scalar.sqrt(sq[:], d1[:])
            rc = gw.tile([128, MB], f32, name="rc")
            nc.vector.reciprocal(rc[:], sq[:])
            mk = gw.tile([128, MB], f32, name="mk")
            nc.vector.tensor_scalar(mk[:], dv_sb[:], 0.0, None, op0=Alu.is_gt)
            iv = gw.tile([128, MB], f32, name="iv")
            nc.vector.tensor_tensor(iv[:], rc[:], mk[:], op=Alu.mult)
            rs0 = gw.tile([128, MB], f32, name="rsu0")
            nc.vector.tensor_scalar_mul(rs0[:], iv[:], inv_de)

            # colscale source: cs = invdv over all N (from dv AllGather)
            q = N // 128
            dvw = gw.tile([128, q], f32, name="dvw")
            nc.sync.dma_start(dvw[:], dv_full.rearrange("(cm p) -> p cm", p=128))
            d1w = gw.tile([128, q], f32, name="d1w")
            nc.vector.tensor_scalar_max(d1w[:], dvw[:], 1.0)
            sqw = gw.tile([128, q], f32, name="sqw")
            nc.scalar.sqrt(sqw[:], d1w[:])
            rcw = gw.tile([128, q], f32, name="rcw")
            nc.vector.reciprocal(rcw[:], sqw[:])
            mkw = gw.tile([128, q], f32, name="mkw")
            nc.vector.tensor_scalar(mkw[:], dvw[:], 0.0, None, op0=Alu.is_gt)
            ivw = gw.tile([128, q], f32, name="ivw")
            nc.vector.tensor_tensor(ivw[:], rcw[:], mkw[:], op=Alu.mult)
            nc.sync.dma_start(cs_dram.rearrange("(cm p) -> p cm", p=128), ivw[:])
            cs2d = cs_dram.rearrange("(a n) -> a n", a=1)

            dgroups = [(0, 1), (2, 3), (4,)]
            for dg in dgroups:
                jds, csts, csjs = {}, {}, {}
                for d in dg:
                    jr = nc.sync.alloc_register(f"jr{d}")
                    nc.sync.reg_load(jr, jsel_sb[0:1, d : d + 1])
                    jds[d] = nc.sync.snap(
                        jr, donate=True, min_val=0, max_val=NCORES - 1
                    )
                    cs_t = gw.tile([1, RP], f32, name="cs_t", tag=f"cs_t{d % 2}")
                    nc.sync.dma_start(cs_t[:], cs2d[:, bass.ds(jds[d] * RP, RP)])
                    csts[d] = cs_t
                for mg in range(2):
                    rhss = {}
                    for mp in range(MB):
                        for d in dg:
                            rh = rhp.tile([128, NCORES, PC], fp16, name="rh", tag="rh")
                            nc.sync.dma_start(
                                rh[:],
                                pk_ag2[mp // 2][:, mp % 2, :, :].rearrange(
                                    "o p c -> p o c"
                                )[:, :, bass.ds(jds[d] * PC, PC)],
                            )
                            rhss[(d, mp)] = rh
                    pss = {}
                    for di, d in enumerate(dg):
                        for t in range(4):
                            pss[(d, t)] = g_ps.tile(
                                [128, 512], f32, name=f"gp{di}{t}", tag=f"gp{di * 4 + t}"
                            )
                    si = 0
                    for mp in range(MB):
                        for o in range(NCORES):
                            s = mp * NCORES + o
                            for d in dg:
                                for t in range(4):
                                    m = mg * 4 + t
                                    mm = nc.tensor.matmul(
                                        pss[(d, t)][:, 0:PC],
                                        lt_all[:, s, m * 128 : (m + 1) * 128],
                                        rhss[(d, mp)][:, o, :],
                                        start=(si == 0),
                                        stop=(si == KK - 1),
                                    )
                                    if o == 0 and t == 0 and d == dg[0]:
                                        _add_dep_helper(
                                            mm.ins, lt_dmas[mp].ins, sync=True,
                                            reason="G after lt load",
                                        )
                            si += 1
                    # evict to SBUF first (frees PSUM banks for csj + next mg)
                    ess = {}
                    for d in dg:
                        for t in range(4):
                            es = esp.tile([128, PC], f32, name="es", tag="es")
                            nc.scalar.copy(es[:], pss[(d, t)][:, 0:PC])
                            ess[(d, t)] = es
                    if mg == 0:
                        for di, d in enumerate(dg):
                            csj = csp.tile([128, PAD], f32, name="csj", tag=f"csj{d % 2}")
                            for hh in range(2):
                                cps = g_ps.tile(
                                    [128, 512], f32, name="cps", tag=f"gp{di * 4 + hh}"
                                )
                                nc.tensor.matmul(
                                    cps[:], ones1[:],
                                    csts[d][:, hh * 512 : (hh + 1) * 512],
                                    start=True, stop=True,
                                )
                                nc.scalar.copy(
                                    csj[:, hh * 512 : (hh + 1) * 512], cps[:]
                                )
                            nc.vector.memset(csj[:, RP:PAD], 0.0)
                            csjs[d] = csj
                    for d in dg:
                        csj3 = csjs[d].rearrange("p (t u) -> p t u", u=3)
                        for t in range(4):
                            m = mg * 4 + t
                            es = ess[(d, t)]
                            z2 = esp.tile([128, PC], f32, name="z2", tag="z2")
                            nc.vector.tensor_scalar(
                                z2[:], es[:], 1.0 / B2, CB2, op0=Alu.mult, op1=Alu.add
                            )
                            m2 = esp.tile([128, PC], f32, name="m2", tag="m2")
                            nc.vector.tensor_scalar(
                                m2[:], z2[:], MAGIC, MAGIC,
                                op0=Alu.add, op1=Alu.subtract,
                            )
                            r2 = esp.tile([128, PC], f32, name="r2", tag="r2")
                            nc.vector.scalar_tensor_tensor(
                                r2[:], in0=m2[:], scalar=-B2, in1=es[:],
                                op0=Alu.mult, op1=Alu.add,
                            )
                            z1 = esp.tile([128, PC], f32, name="z1", tag="z1")
                            nc.vector.tensor_scalar(
                                z1[:], r2[:], 1.0 / BASE, CB1,
                                op0=Alu.mult, op1=Alu.add,
                            )
                            m1 = esp.tile([128, PC], f32, name="m1", tag="m1")
                            nc.vector.tensor_scalar(
                                m1[:], z1[:], MAGIC, MAGIC,
                                op0=Alu.add, op1=Alu.subtract,
                            )
                            m0 = esp.tile([128, PC], f32, name="m0", tag="m0")
                            nc.vector.scalar_tensor_tensor(
                                m0[:], in0=m1[:], scalar=-BASE, in1=r2[:],
                                op0=Alu.mult, op1=Alu.add,
                            )
                            gs = stg.tile([128, PAD], f32, name="gs", tag="gs")
                            gs3 = gs.rearrange("p (t u) -> p t u", u=3)
                            for u, mu in ((2, m2), (1, m1), (0, m0)):
                                au = esp.tile([128, PC], f32, name=f"a{u}", tag="au")
                                nc.scalar.activation(
                                    au[:], mu[:], Act.Copy, scale=rs0[:, m : m + 1]
                                )
                                nc.vector.tensor_tensor(
                                    gs3[:, :, u], au[:], csj3[:, :, u], op=Alu.mult
                                )
                            nc.sync.dma_start(
                                g_out[m * 128 : (m + 1) * 128, d * RP : (d + 1) * RP],
                                gs[:, 0:RP],
                            )

    nc.compile()
    return nc


_CACHE = {}


def get_nc(N, D, KN, NCORES):
    key = (N, D, KN, NCORES)
    if key not in _CACHE:
        _CACHE[key] = build_nc(N, D, KN, NCORES)
    return _CACHE[key]


def kernel(feats, kn, _trace=False):
    feats = np.asarray(feats, dtype=np.float32)
    kn = int(kn)
    N, D = feats.shape
    NCORES = 8
    ND = 5
    RP = N // NCORES
    nc = get_nc(N, D, kn, NCORES)
    ident = np.eye(128, dtype=np.float32)
    in_maps = []
    for c in range(NCORES):
        jsel = np.zeros((1, 8), np.int32)
        for d in range(8):
            jsel[0, d] = (c + d) % NCORES
        in_maps.append({
            "feats_all": feats,
            "feats_my": feats[c * RP : (c + 1) * RP],
            "ident_in": ident,
            "jsel_in": jsel,
        })
    res = run_bass_kernel_spmd(
        nc, in_maps, core_ids=list(range(NCORES)), trace=_trace
    )
    out = np.empty((N, N), dtype=np.float32)
    for c in range(NCORES):
        g = res.results[c]["g_out"]  # [RP, ND*RP]
        for d in range(ND):
            j = (c + d) % NCORES
            if d == 4 and c >= 4:
                continue
            blk = g[:, d * RP : (d + 1) * RP]
            out[c * RP : (c + 1) * RP, j * RP : (j + 1) * RP] = blk
            if d != 0:
                out[j * RP : (j + 1) * RP, c * RP : (c + 1) * RP] = blk.T
    if _trace:
        return out, res
    return out


if __name__ == "__main__":
    inputs = {
        "feats": np.load("/tmp/feats.npy"),
        "kn": 10,
    }
    out = kernel(**inputs)
    print("out", out.shape, out.dtype, float(np.abs(out).max()))


# revision 24
# speedup vs baseline: 1.0367x; 1.0367x over previous
"""Trainium2 Bass kernel for nn_Attention_16011638079620 (gnn_message_passing).

Computes, for feats [8192, 256] f32 and kn=10:
    sim   = cosine-similarity(feats)            [N, N]
    B     = rowwise top-kn one-hot mask of softmax(sim) (rank-preserving)
    G     = (1/kn) * invdv_i * invdv_j * (B^T B)_ij,  dv = colsums of B

Strategy (8 cores):
  - sim via 3-pass bf16 hi/lo split matmuls (exact top-k, 4x faster than f32)
  - B columns packed 3-per-fp16 value (base 24; counts <= 23 so the packed
    matmul B^T @ packedB is integer-exact in fp32 PSUM) -> 1.5x fp8-DR rate
  - G is symmetric: core c computes blocks (c, (c+d)%8) for d=0..4; host
    mirrors the rest. lhsT is always the core's own column slice (AllToAll).
  - dv via fp8-DoubleRow ones-matmuls, interleaved with the sim phase.
  - software pipeline: mask/pack of block m-1 runs behind sim of block m so
    PSUM banks release early; collectives merged in block pairs.
"""

import sys

sys.path.insert(0, "/opt/trn_rl_repo")

from contextlib import ExitStack

import numpy as np

import concourse.bass as bass
import concourse.tile as tile
from concourse import bacc, mybir
from concourse.bass import _add_dep_helper
from concourse.bass_utils import run_bass_kernel_spmd

f32 = mybir.dt.float32
bf16 = mybir.dt.bfloat16
fp16 = mybir.dt.float16
fp8 = mybir.dt.float8e4
i32 = mybir.dt.int32
Alu = mybir.AluOpType
Act = mybir.ActivationFunctionType
NEG = -1e30
BASE = 24.0
B2 = BASE * BASE  # 576
MAGIC = 12582912.0  # 1.5 * 2**23: (z + MAGIC) - MAGIC == round-to-nearest(z)
CB2 = 0.5 / B2 - 0.5  # bias so round(P/576 + CB2) == floor(P/576) exactly
CB1 = 0.5 / BASE - 0.5


def build_nc(N, D, KN, NCORES):
    RP = N // NCORES           # 1024 rows/G-rows per core
    MB = RP // 128             # 8 row blocks per core
    NCH = N // 512             # topk chunks
    DT = D // 128              # 2 feature chunks
    PC = 342                   # packed cols per 1024-col slice (342*3 = 1026)
    PCW = NCORES * PC          # 2736 packed cols total per row
    PAD = 3 * PC               # 1026 padded cols per slice
    BW = NCORES * PAD          # 8208 padded mask width
    KK = N // 128              # 64 contraction chunks for phase D
    NP = MB // 2               # 4 block pairs for collectives
    ND = 5                     # symmetric blocks per core
    assert 8 < KN <= 16

    inv_de = float(np.float32(1.0) / np.float32(KN))

    nc = bacc.Bacc(
        "TRN2",
        target_bir_lowering=False,
        debug=False,
        enable_asserts=False,
        num_devices=NCORES,
    )
    feats_all = nc.dram_tensor("feats_all", [N, D], f32, kind="ExternalInput").ap()
    feats_my = nc.dram_tensor("feats_my", [RP, D], f32, kind="ExternalInput").ap()
    ident_in = nc.dram_tensor("ident_in", [128, 128], f32, kind="ExternalInput").ap()
    jsel_in = nc.dram_tensor("jsel_in", [1, 8], i32, kind="ExternalInput").ap()
    g_out = nc.dram_tensor("g_out", [RP, ND * RP], f32, kind="ExternalOutput").ap()

    rg = [list(range(NCORES))]

    with tile.TileContext(nc) as tc, ExitStack() as ctx:
        dram = ctx.enter_context(tc.tile_pool(name="dram", bufs=1, space="DRAM"))
        b_grp2 = [
            dram.tile([NCORES, 2, 128, RP], fp8, name=f"b_grp2_{p}") for p in range(NP)
        ]
        lts_d2 = [
            dram.tile([NCORES, 2, 128, RP], fp8, name=f"lts_d2_{p}") for p in range(NP)
        ]
        pk_in2 = [dram.tile([2, 128, PCW], fp16, name=f"pk_in2_{p}") for p in range(NP)]
        pk_ag2 = [
            dram.tile(
                [NCORES, 2, 128, PCW], fp16, addr_space="Shared", name=f"pk_ag2_{p}"
            )
            for p in range(NP)
        ]
        dv_my_d = dram.tile([RP], f32, name="dv_my_d")
        dv_full = dram.tile([N], f32, addr_space="Shared", name="dv_full")
        cs_dram = dram.tile([N], f32, name="cs_dram")

        pers = ctx.enter_context(tc.tile_pool(name="pers", bufs=1))
        dv_stack = ExitStack()
        dvps_pool = dv_stack.enter_context(
            tc.tile_pool(name="dvps", bufs=1, space="PSUM")
        )
        dv_ps = dvps_pool.tile([128, MB], f32, name="dv_ps")

        ident = pers.tile([128, 128], f32, name="ident")
        nc.sync.dma_start(ident[:], ident_in)
        idb = pers.tile([128, 128], bf16, name="idb")
        nc.vector.tensor_copy(idb[:], ident[:])
        jsel_sb = pers.tile([1, 8], i32, name="jsel_sb")
        nc.sync.dma_start(jsel_sb[:], jsel_in)
        ones1 = pers.tile([1, 128], f32, name="ones1")
        nc.vector.memset(ones1[:], 1.0)
        ones_dr = pers.tile([128, 2, 16], fp8, name="ones_dr")
        nc.vector.memset(ones_dr[:], 1.0)

        # lhsT for phase D: [128, slot = m'*NCORES+o, my 1024 cols] fp8 (8MB)
        lt_all = pers.tile([128, KK, RP], fp8, name="lt_all")

        # ---------------- phase 1: normalize + hi/lo split + transpose ------
        with ExitStack() as p12:
            fsb = p12.enter_context(tc.tile_pool(name="fsb", bufs=1))
            fnt_hi = [fsb.tile([128, N], bf16, name=f"fh{h}") for h in range(DT)]
            fnt_lo = [fsb.tile([128, N], bf16, name=f"fl{h}") for h in range(DT)]
            fnt_myh = [fsb.tile([128, RP], bf16, name=f"fmh{h}") for h in range(DT)]
            fnt_myl = [fsb.tile([128, RP], bf16, name=f"fml{h}") for h in range(DT)]

            with ExitStack() as p1:
                wrk = p1.enter_context(tc.tile_pool(name="wrk", bufs=3))
                sml = p1.enter_context(tc.tile_pool(name="sml", bufs=6))
                tp_ps = p1.enter_context(
                    tc.tile_pool(name="tp_ps", bufs=1, space="PSUM")
                )

                def norm_group(src4, dh, dl, col0, nb):
                    # nb row-blocks batched: one op set for the whole group
                    ft4 = wrk.tile([128, nb, D], f32, name="ft4")
                    nc.sync.dma_start(ft4[:], src4)
                    tps = {}
                    for x in range(2):
                        for h in range(DT):
                            tps[(x, h)] = tp_ps.tile(
                                [128, nb * 128], bf16, name=f"tp{x}{h}", tag=f"tp{x}{h}"
                            )
                    sq4 = wrk.tile([128, nb, D], f32, name="sq4")
                    nc.scalar.square(
                        sq4.rearrange("p b d -> p (b d)"),
                        ft4.rearrange("p b d -> p (b d)"),
                    )
                    n24 = sml.tile([128, nb, 1], f32, name="n24")
                    nc.vector.reduce_sum(n24[:], sq4[:], axis=mybir.AxisListType.X)
                    nrm4 = sml.tile([128, nb, 1], f32, name="nrm4")
                    nc.scalar.sqrt(
                        nrm4.rearrange("p b o -> p (b o)"),
                        n24.rearrange("p b o -> p (b o)"),
                    )
                    inv4 = sml.tile([128, nb, 1], f32, name="inv4")
                    nc.vector.reciprocal(
                        inv4.rearrange("p b o -> p (b o)"),
                        nrm4.rearrange("p b o -> p (b o)"),
                    )
                    fn4 = wrk.tile([128, nb, D], f32, name="fn4")
                    nc.vector.tensor_tensor(
                        fn4[:], ft4[:], inv4[:].broadcast_to([128, nb, D]),
                        op=Alu.mult,
                    )
                    fh4 = wrk.tile([128, nb, D], bf16, name="fh4")
                    nc.scalar.copy(
                        fh4.rearrange("p b d -> p (b d)"),
                        fn4.rearrange("p b d -> p (b d)"),
                    )
                    fl4 = wrk.tile([128, nb, D], bf16, name="fl4")
                    nc.vector.tensor_tensor(
                        fl4[:], fn4[:], fh4[:], op=Alu.subtract
                    )
                    for i in range(nb):
                        for h in range(DT):
                            for x, src in ((0, fh4), (1, fl4)):
                                nc.tensor.transpose(
                                    tps[(x, h)][:, i * 128 : (i + 1) * 128],
                                    src[:, i, h * 128 : (h + 1) * 128],
                                    idb[:],
                                )
                    for h in range(DT):
                        for x, dst in ((0, dh), (1, dl)):
                            nc.scalar.copy(
                                dst[h][:, col0 : col0 + nb * 128], tps[(x, h)][:]
                            )

                fm4 = feats_my.rearrange("(g i p) d -> g p i d", p=128, i=4)
                for g in range(MB // 4):
                    norm_group(fm4[g], fnt_myh, fnt_myl, g * 512, 4)
                fa4 = feats_all.rearrange("(g i p) d -> g p i d", p=128, i=4)
                for g in range(N // 512):
                    norm_group(fa4[g], fnt_hi, fnt_lo, g * 512, 4)

            # ---------------- phase 2: sim, topk, mask, pack, CC -----------
            with ExitStack() as p2:
                simp = p2.enter_context(tc.tile_pool(name="simp", bufs=3))
                smal = p2.enter_context(tc.tile_pool(name="smal", bufs=2))
                bmpp = p2.enter_context(tc.tile_pool(name="bmpp", bufs=1))
                pkp = p2.enter_context(tc.tile_pool(name="pkp", bufs=1))
                t0p = p2.enter_context(tc.tile_pool(name="t0p", bufs=2))
                sim_ps = p2.enter_context(
                    tc.tile_pool(name="sim_ps", bufs=1, space="PSUM")
                )
                combos = []
                for h in range(DT):
                    combos.append((fnt_myh[h], fnt_hi[h]))
                    combos.append((fnt_myh[h], fnt_lo[h]))
                    combos.append((fnt_myl[h], fnt_hi[h]))

                tkns = {}
                halves = {}

                def sim_block(m):
                    sh0 = simp.tile([128, N // 2], f32, name="sh0", tag="sh")
                    sh1 = simp.tile([128, N // 2], f32, name="sh1", tag="sh")
                    halves[m] = (sh0, sh1)
                    cand = smal.tile([128, 8 * NCH], f32, name="cand", tag="cand")
                    for qr in range(4):
                        pss = [
                            sim_ps.tile([128, 512], f32, name=f"sq{t}", tag=f"sq{t}")
                            for t in range(4)
                        ]
                        for ci, (la, ra) in enumerate(combos):
                            lt = la[:, m * 128 : (m + 1) * 128]
                            for t in range(4):
                                ntc = qr * 4 + t
                                nc.tensor.matmul(
                                    pss[t][:],
                                    lt,
                                    ra[:, ntc * 512 : (ntc + 1) * 512],
                                    start=(ci == 0),
                                    stop=(ci == 5),
                                )
                        sh = (sh0, sh1)[qr // 2]
                        for t in range(4):
                            ntc = qr * 4 + t
                            nc.vector.max(
                                cand[:, ntc * 8 : (ntc + 1) * 8], pss[t][:]
                            )
                            nc.scalar.copy(
                                sh[:, (ntc % 8) * 512 : (ntc % 8 + 1) * 512],
                                pss[t][:],
                            )
                    c8 = smal.tile([128, 8], f32, name="c8", tag="c8")
                    nc.vector.max(c8[:], cand[:])
                    cand2 = smal.tile([128, 8 * NCH], f32, name="cand2", tag="cand2")
                    nc.vector.match_replace(cand2[:], c8[:], cand[:], NEG)
                    c8b = smal.tile([128, 8], f32, name="c8b", tag="c8b")
                    nc.vector.max(c8b[:], cand2[:])
                    tkns[m] = c8b

                def mask_pack(m):
                    tkn = tkns[m][:, KN - 9 : KN - 8]
                    sh0, sh1 = halves[m]
                    bmp = bmpp.tile([128, BW], fp8, name="bmp")
                    for j in range(NCORES):
                        sh = (sh0, sh1)[j // 4]
                        nc.vector.tensor_scalar(
                            bmp[:, j * PAD : j * PAD + RP],
                            sh[:, (j % 4) * RP : (j % 4 + 1) * RP],
                            tkn,
                            None,
                            op0=Alu.is_ge,
                        )
                        nc.vector.memset(bmp[:, j * PAD + RP : (j + 1) * PAD], 0.0)
                    pk = pkp.tile([128, PCW], fp16, name="pk")
                    bm3 = bmp.rearrange("p (j t u) -> p j t u", j=NCORES, u=3)
                    for j in range(NCORES):
                        t0 = t0p.tile([128, PC], f32, name="t0")
                        nc.vector.scalar_tensor_tensor(
                            t0[:], in0=bm3[:, j, :, 1], scalar=BASE,
                            in1=bm3[:, j, :, 0], op0=Alu.mult, op1=Alu.add,
                        )
                        nc.vector.scalar_tensor_tensor(
                            pk[:, j * PC : (j + 1) * PC],
                            in0=bm3[:, j, :, 2], scalar=B2, in1=t0[:],
                            op0=Alu.mult, op1=Alu.add,
                        )
                    pp, i = m // 2, m % 2
                    nc.sync.dma_start(pk_in2[pp][i], pk[:])
                    nc.sync.dma_start(
                        b_grp2[pp][:, i].rearrange("j p q -> p j q"),
                        bmp.rearrange("p (j q) -> p j q", j=NCORES)[:, :, 0:RP],
                    )

                lt_dma = {}

                def cc_pair(pp):
                    nc.gpsimd.collective_compute(
                        "AllToAll", Alu.bypass, replica_groups=rg,
                        ins=[b_grp2[pp].opt()], outs=[lts_d2[pp].opt()],
                    )
                    nc.gpsimd.collective_compute(
                        "AllGather", Alu.bypass, replica_groups=rg,
                        ins=[pk_in2[pp].opt()], outs=[pk_ag2[pp].opt()],
                    )
                    for i in range(2):
                        mp = 2 * pp + i
                        lt_dma[mp] = nc.sync.dma_start(
                            lt_all[:, mp * NCORES : (mp + 1) * NCORES, :],
                            lts_d2[pp][:, i].rearrange("o p q -> p o q"),
                        )

                ltp = lt_all.rearrange("p (t i o) q -> p t i o q", i=2, o=NCORES)

                def dv_pair(pp):
                    first = True
                    for o in range(NCORES):
                        for m in range(MB):
                            mm = nc.tensor.matmul(
                                dv_ps[:, m : m + 1],
                                ltp[:, pp, :, o, m * 128 : (m + 1) * 128],
                                ones_dr[:, :, 0:1],
                                perf_mode=mybir.MatmulPerfMode.DoubleRow,
                                start=(pp == 0 and o == 0 and m == 0),
                                stop=(pp == NP - 1 and o == NCORES - 1 and m == MB - 1),
                                skip_group_check=True,
                            )
                            if first:
                                for mp in (2 * pp, 2 * pp + 1):
                                    _add_dep_helper(
                                        mm.ins, lt_dma[mp].ins, sync=True,
                                        reason="dv after lt load",
                                    )
                                first = False

                for m in range(MB):
                    sim_block(m)
                    if m >= 1:
                        mask_pack(m - 1)
                        if (m - 1) % 2 == 1:
                            cc_pair((m - 1) // 2)
                    if m >= 4 and m % 2 == 0:
                        dv_pair((m - 4) // 2)
                mask_pack(MB - 1)
                cc_pair(NP - 1)
                dv_pair(NP - 2)
                dv_pair(NP - 1)
                lt_dmas = lt_dma

        # ---------------- phase 3+4: dv scales, G blocks --------------------
        with ExitStack() as p4:
            gw = p4.enter_context(tc.tile_pool(name="gw", bufs=4))
            dv_sb = gw.tile([128, MB], f32, name="dv_sb")
            nc.vector.tensor_copy(dv_sb[:], dv_ps[:])
            dv_stack.close()
            csp = p4.enter_context(tc.tile_pool(name="csp", bufs=2))
            stg = p4.enter_context(tc.tile_pool(name="stg", bufs=2))
            esp = p4.enter_context(tc.tile_pool(name="esp", bufs=2))
            rhp = p4.enter_context(tc.tile_pool(name="rhp", bufs=MB + 1))
            g_ps = p4.enter_context(tc.tile_pool(name="g_ps", bufs=1, space="PSUM"))
            nc.scalar.dma_start(dv_my_d.rearrange("(m p) -> p m", p=128), dv_sb[:])
            nc.gpsimd.collective_compute(
                "AllGather", Alu.bypass, replica_groups=rg,
                ins=[dv_my_d.opt()], outs=[dv_full.opt()],
            )
            # rowscale: rs = invdv(my col block) * inv_de
            d1 = gw.tile([128, MB], f32, name="d1")
            nc.vector.tensor_scalar_max(d1[:], dv_sb[:], 1.0)
            sq = gw.tile([128, MB], f32, name="sqv")
            nc.scalar.sqrt(sq[:], d1[:])
            rc = gw.tile([128, MB], f32, name="rc")
            nc.vector.reciprocal(rc[:], sq[:])
            mk = gw.tile([128, MB], f32, name="mk")
            nc.vector.tensor_scalar(mk[:], dv_sb[:], 0.0, None, op0=Alu.is_gt)
            iv = gw.tile([128, MB], f32, name="iv")
            nc.vector.tensor_tensor(iv[:], rc[:], mk[:], op=Alu.mult)
            rs0 = gw.tile([128, MB], f32, name="rsu0")
            nc.vector.tensor_scalar_mul(rs0[:], iv[:], inv_de)

            # colscale source: cs = invdv over all N (from dv AllGather)
            q = N // 128
            dvw = gw.tile([128, q], f32, name="dvw")
            nc.scalar.dma_start(dvw[:], dv_full.rearrange("(cm p) -> p cm", p=128))
            d1w = gw.tile([128, q], f32, name="d1w")
            nc.vector.tensor_scalar_max(d1w[:], dvw[:], 1.0)
            sqw = gw.tile([128, q], f32, name="sqw")
            nc.scalar.sqrt(sqw[:], d1w[:])
            rcw = gw.tile([128, q], f32, name="rcw")
            nc.vector.reciprocal(rcw[:], sqw[:])
            mkw = gw.tile([128, q], f32, name="mkw")
            nc.vector.tensor_scalar(mkw[:], dvw[:], 0.0, None, op0=Alu.is_gt)
            ivw = gw.tile([128, q], f32, name="ivw")
            nc.vector.tensor_tensor(ivw[:], rcw[:], mkw[:], op=Alu.mult)
            nc.scalar.dma_start(cs_dram.rearrange("(cm p) -> p cm", p=128), ivw[:])
            cs2d = cs_dram.rearrange("(a n) -> a n", a=1)

            dgroups = [(0, 1), (2, 3), (4,)]
            for dg in dgroups:
                jds, csts, csjs = {}, {}, {}
                for d in dg:
                    jr = nc.sync.alloc_register(f"jr{d}")
                    nc.sync.reg_load(jr, jsel_sb[0:1, d : d + 1])
                    jds[d] = nc.sync.snap(
                        jr, donate=True, min_val=0, max_val=NCORES - 1
                    )
                    jra = nc.scalar.alloc_register(f"jra{d}")
                    nc.scalar.reg_load(jra, jsel_sb[0:1, d : d + 1])
                    jda = nc.scalar.snap(
                        jra, donate=True, min_val=0, max_val=NCORES - 1
                    )
                    cs_t = gw.tile([1, RP], f32, name="cs_t", tag=f"cs_t{d % 2}")
                    nc.scalar.dma_start(cs_t[:], cs2d[:, bass.ds(jda * RP, RP)])
                    csts[d] = cs_t
                for mg in range(2):
                    rhss = {}
                    for mp in range(MB):
                        for d in dg:
                            rh = rhp.tile([128, NCORES, PC], fp16, name="rh", tag="rh")
                            nc.sync.dma_start(
                                rh[:],
                                pk_ag2[mp // 2][:, mp % 2, :, :].rearrange(
                                    "o p c -> p o c"
                                )[:, :, bass.ds(jds[d] * PC, PC)],
                            )
                            rhss[(d, mp)] = rh
                    pss = {}
                    for di, d in enumerate(dg):
                        for t in range(4):
                            pss[(d, t)] = g_ps.tile(
                                [128, 512], f32, name=f"gp{di}{t}", tag=f"gp{di * 4 + t}"
                            )
                    si = 0
                    for mp in range(MB):
                        for o in range(NCORES):
                            s = mp * NCORES + o
                            for d in dg:
                                for t in range(4):
                                    m = mg * 4 + t
                                    mm = nc.tensor.matmul(
                                        pss[(d, t)][:, 0:PC],
                                        lt_all[:, s, m * 128 : (m + 1) * 128],
                                        rhss[(d, mp)][:, o, :],
                                        start=(si == 0),
                                        stop=(si == KK - 1),
                                    )
                                    if o == 0 and t == 0 and d == dg[0]:
                                        _add_dep_helper(
                                            mm.ins, lt_dmas[mp].ins, sync=True,
                                            reason="G after lt load",
                                        )
                            si += 1
                    # evict to SBUF first (frees PSUM banks for csj + next mg)
                    ess = {}
                    for d in dg:
                        for t in range(4):
                            es = esp.tile([128, PC], f32, name="es", tag="es")
                            nc.scalar.copy(es[:], pss[(d, t)][:, 0:PC])
                            ess[(d, t)] = es
                    if mg == 0:
                        for di, d in enumerate(dg):
                            csj = csp.tile([128, PAD], f32, name="csj", tag=f"csj{d % 2}")
                            for hh in range(2):
                                cps = g_ps.tile(
                                    [128, 512], f32, name="cps", tag=f"gp{di * 4 + hh}"
                                )
                                nc.tensor.matmul(
                                    cps[:], ones1[:],
                                    csts[d][:, hh * 512 : (hh + 1) * 512],
                                    start=True, stop=True,
                                )
                                nc.scalar.copy(
                                    csj[:, hh * 512 : (hh + 1) * 512], cps[:]
                                )
                            nc.vector.memset(csj[:, RP:PAD], 0.0)
                            csjs[d] = csj
                    for d in dg:
                        csj3 = csjs[d].rearrange("p (t u) -> p t u", u=3)
                        for t in range(4):
                            m = mg * 4 + t
                            es = ess[(d, t)]
                            z2 = esp.tile([128, PC], f32, name="z2", tag="z2")
                            nc.vector.tensor_scalar(
                                z2[:], es[:], 1.0 / B2, CB2, op0=Alu.mult, op1=Alu.add
                            )
                            m2 = esp.tile([128, PC], f32, name="m2", tag="m2")
                            nc.vector.tensor_scalar(
                                m2[:], z2[:], MAGIC, MAGIC,
                                op0=Alu.add, op1=Alu.subtract,
                            )
                            r2 = esp.tile([128, PC], f32, name="r2", tag="r2")
                            nc.vector.scalar_tensor_tensor(
                                r2[:], in0=m2[:], scalar=-B2, in1=es[:],
                                op0=Alu.mult, op1=Alu.add,
                            )
                            z1 = esp.tile([128, PC], f32, name="z1", tag="z1")
                            nc.vector.tensor_scalar(
                                z1[:], r2[:], 1.0 / BASE, CB1,
                                op0=Alu.mult, op1=Alu.add,
                            )
                            m1 = esp.tile([128, PC], f32, name="m1", tag="m1")
                            nc.vector.tensor_scalar(
                                m1[:], z1[:], MAGIC, MAGIC,
                                op0=Alu.add, op1=Alu.subtract,
                            )
                            m0 = esp.tile([128, PC], f32, name="m0", tag="m0")
                            nc.vector.scalar_tensor_tensor(
                                m0[:], in0=m1[:], scalar=-BASE, in1=r2[:],
                                op0=Alu.mult, op1=Alu.add,
                            )
                            gs = stg.tile([128, PAD], f32, name="gs", tag="gs")
                            gs3 = gs.rearrange("p (t u) -> p t u", u=3)
                            for u, mu in ((2, m2), (1, m1), (0, m0)):
                                au = esp.tile([128, PC], f32, name=f"a{u}", tag="au")
                                nc.scalar.activation(
                                    au[:], mu[:], Act.Copy, scale=rs0[:, m : m + 1]
                                )
                                nc.vector.tensor_tensor(
                                    gs3[:, :, u], au[:], csj3[:, :, u], op=Alu.mult
                                )
                            nc.sync.dma_start(
                                g_out[m * 128 : (m + 1) * 128, d * RP : (d + 1) * RP],
                                gs[:, 0:RP],
                            )

    nc.compile()
    return nc


_CACHE = {}


def get_nc(N, D, KN, NCORES):
    key = (N, D, KN, NCORES)
    if key not in _CACHE:
        _CACHE[key] = build_nc(N, D, KN, NCORES)
    return _CACHE[key]


def kernel(feats, kn, _trace=False):
    feats = np.asarray(feats, dtype=np.float32)
    kn = int(kn)
    N, D = feats.shape
    NCORES = 8
    ND = 5
    RP = N // NCORES
    nc = get_nc(N, D, kn, NCORES)
    ident = np.eye(128, dtype=np.float32)
    in_maps = []
    for c in range(NCORES):
        jsel = np.zeros((1, 8), np.int32)
        for d in range(8):
            jsel[0, d] = (c + d) % NCORES
        in_maps.append({
            "feats_all": feats,
            "feats_my": feats[c * RP : (c + 1) * RP],
            "ident_in": ident,
            "jsel_in": jsel,
        })
    res = run_bass_kernel_spmd(
        nc, in_maps, core_ids=list(range(NCORES)), trace=_trace
    )
    out = np.empty((N, N), dtype=np.float32)
    for c in range(NCORES):
        g = res.results[c]["g_out"]  # [RP, ND*RP]
        for d in range(ND):
            j = (c + d) % NCORES
            if d == 4 and c >= 4:
                continue
            blk = g[:, d * RP : (d + 1) * RP]
            out[c * RP : (c + 1) * RP, j * RP : (j + 1) * RP] = blk
            if d != 0:
                out[j * RP : (j + 1) * RP, c * RP : (c + 1) * RP] = blk.T
    if _trace:
        return out, res
    return out


if __name__ == "__main__":
    inputs = {
        "feats": np.load("/tmp/feats.npy"),
        "kn": 10,
    }
    out = kernel(**inputs)
    print("out", out.shape, out.dtype, float(np.abs(out).max()))


# revision 25
# speedup vs baseline: 1.0552x; 1.0178x over previous
"""Trainium2 Bass kernel for nn_Attention_16011638079620 (gnn_message_passing).

Computes, for feats [8192, 256] f32 and kn=10:
    sim   = cosine-similarity(feats)            [N, N]
    B     = rowwise top-kn one-hot mask of softmax(sim) (rank-preserving)
    G     = (1/kn) * invdv_i * invdv_j * (B^T B)_ij,  dv = colsums of B

Strategy (8 cores):
  - sim via 3-pass bf16 hi/lo split matmuls (exact top-k, 4x faster than f32)
  - B columns packed 3-per-fp16 value (base 24; counts <= 23 so the packed
    matmul B^T @ packedB is integer-exact in fp32 PSUM) -> 1.5x fp8-DR rate
  - G is symmetric: core c computes blocks (c, (c+d)%8) for d=0..4; host
    mirrors the rest. lhsT is always the core's own column slice (AllToAll).
  - dv via fp8-DoubleRow ones-matmuls, interleaved with the sim phase.
  - software pipeline: mask/pack of block m-1 runs behind sim of block m so
    PSUM banks release early; collectives merged in block pairs.
"""

import sys

sys.path.insert(0, "/opt/trn_rl_repo")

from contextlib import ExitStack

import numpy as np

import concourse.bass as bass
import concourse.tile as tile
from concourse import bacc, mybir
from concourse.bass import _add_dep_helper
from concourse.bass_utils import run_bass_kernel_spmd

f32 = mybir.dt.float32
bf16 = mybir.dt.bfloat16
fp16 = mybir.dt.float16
fp8 = mybir.dt.float8e4
i32 = mybir.dt.int32
Alu = mybir.AluOpType
Act = mybir.ActivationFunctionType
NEG = -1e30
BASE = 24.0
B2 = BASE * BASE  # 576
MAGIC = 12582912.0  # 1.5 * 2**23: (z + MAGIC) - MAGIC == round-to-nearest(z)
CB2 = 0.5 / B2 - 0.5  # bias so round(P/576 + CB2) == floor(P/576) exactly
CB1 = 0.5 / BASE - 0.5


def build_nc(N, D, KN, NCORES):
    RP = N // NCORES           # 1024 rows/G-rows per core
    MB = RP // 128             # 8 row blocks per core
    NCH = N // 512             # topk chunks
    DT = D // 128              # 2 feature chunks
    PC = 342                   # packed cols per 1024-col slice (342*3 = 1026)
    PCW = NCORES * PC          # 2736 packed cols total per row
    PAD = 3 * PC               # 1026 padded cols per slice
    BW = NCORES * PAD          # 8208 padded mask width
    KK = N // 128              # 64 contraction chunks for phase D
    NP = MB // 2               # 4 block pairs for collectives
    ND = 5                     # symmetric blocks per core
    assert 8 < KN <= 16

    inv_de = float(np.float32(1.0) / np.float32(KN))

    nc = bacc.Bacc(
        "TRN2",
        target_bir_lowering=False,
        debug=False,
        enable_asserts=False,
        num_devices=NCORES,
    )
    feats_all = nc.dram_tensor("feats_all", [N, D], f32, kind="ExternalInput").ap()
    feats_my = nc.dram_tensor("feats_my", [RP, D], f32, kind="ExternalInput").ap()
    ident_in = nc.dram_tensor("ident_in", [128, 128], f32, kind="ExternalInput").ap()
    jsel_in = nc.dram_tensor("jsel_in", [1, 8], i32, kind="ExternalInput").ap()
    g_out = nc.dram_tensor("g_out", [RP, ND * RP], f32, kind="ExternalOutput").ap()

    rg = [list(range(NCORES))]

    with tile.TileContext(nc) as tc, ExitStack() as ctx:
        dram = ctx.enter_context(tc.tile_pool(name="dram", bufs=1, space="DRAM"))
        b_grp2 = [
            dram.tile([NCORES, 2, 128, RP], fp8, name=f"b_grp2_{p}") for p in range(NP)
        ]
        lts_d2 = [
            dram.tile([NCORES, 2, 128, RP], fp8, name=f"lts_d2_{p}") for p in range(NP)
        ]
        pk_in2 = [dram.tile([2, 128, PCW], fp16, name=f"pk_in2_{p}") for p in range(NP)]
        pk_ag2 = [
            dram.tile(
                [NCORES, 2, 128, PCW], fp16, addr_space="Shared", name=f"pk_ag2_{p}"
            )
            for p in range(NP)
        ]
        dv_my_d = dram.tile([RP], f32, name="dv_my_d")
        dv_full = dram.tile([N], f32, addr_space="Shared", name="dv_full")
        cs_dram = dram.tile([N], f32, name="cs_dram")

        pers = ctx.enter_context(tc.tile_pool(name="pers", bufs=1))
        dv_stack = ExitStack()
        dvps_pool = dv_stack.enter_context(
            tc.tile_pool(name="dvps", bufs=1, space="PSUM")
        )
        dv_ps = dvps_pool.tile([128, MB], f32, name="dv_ps")

        ident = pers.tile([128, 128], f32, name="ident")
        nc.sync.dma_start(ident[:], ident_in)
        idb = pers.tile([128, 128], bf16, name="idb")
        nc.vector.tensor_copy(idb[:], ident[:])
        jsel_sb = pers.tile([1, 8], i32, name="jsel_sb")
        nc.sync.dma_start(jsel_sb[:], jsel_in)
        ones1 = pers.tile([1, 128], f32, name="ones1")
        nc.vector.memset(ones1[:], 1.0)
        ones_dr = pers.tile([128, 2, 16], fp8, name="ones_dr")
        nc.vector.memset(ones_dr[:], 1.0)

        # lhsT for phase D: [128, slot = m'*NCORES+o, my 1024 cols] fp8 (8MB)
        lt_all = pers.tile([128, KK, RP], fp8, name="lt_all")

        # ---------------- phase 1: normalize + hi/lo split + transpose ------
        with ExitStack() as p12:
            fsb = p12.enter_context(tc.tile_pool(name="fsb", bufs=1))
            fnt_hi = [fsb.tile([128, N], bf16, name=f"fh{h}") for h in range(DT)]
            fnt_lo = [fsb.tile([128, N], bf16, name=f"fl{h}") for h in range(DT)]
            fnt_myh = [fsb.tile([128, RP], bf16, name=f"fmh{h}") for h in range(DT)]
            fnt_myl = [fsb.tile([128, RP], bf16, name=f"fml{h}") for h in range(DT)]

            with ExitStack() as p1:
                wrk = p1.enter_context(tc.tile_pool(name="wrk", bufs=3))
                sml = p1.enter_context(tc.tile_pool(name="sml", bufs=6))
                tp_ps = p1.enter_context(
                    tc.tile_pool(name="tp_ps", bufs=1, space="PSUM")
                )

                def norm_group(src4, dh, dl, col0, nb):
                    # nb row-blocks batched: one op set for the whole group
                    ft4 = wrk.tile([128, nb, D], f32, name="ft4")
                    nc.sync.dma_start(ft4[:], src4)
                    tps = {}
                    for x in range(2):
                        for h in range(DT):
                            tps[(x, h)] = tp_ps.tile(
                                [128, nb * 128], bf16, name=f"tp{x}{h}", tag=f"tp{x}{h}"
                            )
                    sq4 = wrk.tile([128, nb, D], f32, name="sq4")
                    nc.scalar.square(
                        sq4.rearrange("p b d -> p (b d)"),
                        ft4.rearrange("p b d -> p (b d)"),
                    )
                    n24 = sml.tile([128, nb, 1], f32, name="n24")
                    nc.vector.reduce_sum(n24[:], sq4[:], axis=mybir.AxisListType.X)
                    nrm4 = sml.tile([128, nb, 1], f32, name="nrm4")
                    nc.scalar.sqrt(
                        nrm4.rearrange("p b o -> p (b o)"),
                        n24.rearrange("p b o -> p (b o)"),
                    )
                    inv4 = sml.tile([128, nb, 1], f32, name="inv4")
                    nc.vector.reciprocal(
                        inv4.rearrange("p b o -> p (b o)"),
                        nrm4.rearrange("p b o -> p (b o)"),
                    )
                    fn4 = wrk.tile([128, nb, D], f32, name="fn4")
                    nc.vector.tensor_tensor(
                        fn4[:], ft4[:], inv4[:].broadcast_to([128, nb, D]),
                        op=Alu.mult,
                    )
                    fh4 = wrk.tile([128, nb, D], bf16, name="fh4")
                    nc.scalar.copy(
                        fh4.rearrange("p b d -> p (b d)"),
                        fn4.rearrange("p b d -> p (b d)"),
                    )
                    fl4 = wrk.tile([128, nb, D], bf16, name="fl4")
                    nc.vector.tensor_tensor(
                        fl4[:], fn4[:], fh4[:], op=Alu.subtract
                    )
                    for i in range(nb):
                        for h in range(DT):
                            for x, src in ((0, fh4), (1, fl4)):
                                nc.tensor.transpose(
                                    tps[(x, h)][:, i * 128 : (i + 1) * 128],
                                    src[:, i, h * 128 : (h + 1) * 128],
                                    idb[:],
                                )
                    for h in range(DT):
                        for x, dst in ((0, dh), (1, dl)):
                            nc.scalar.copy(
                                dst[h][:, col0 : col0 + nb * 128], tps[(x, h)][:]
                            )

                fm4 = feats_my.rearrange("(g i p) d -> g p i d", p=128, i=4)
                for g in range(MB // 4):
                    norm_group(fm4[g], fnt_myh, fnt_myl, g * 512, 4)
                fa4 = feats_all.rearrange("(g i p) d -> g p i d", p=128, i=4)
                for g in range(N // 512):
                    norm_group(fa4[g], fnt_hi, fnt_lo, g * 512, 4)

            # ---------------- phase 2: sim, topk, mask, pack, CC -----------
            with ExitStack() as p2:
                simp = p2.enter_context(tc.tile_pool(name="simp", bufs=3))
                smal = p2.enter_context(tc.tile_pool(name="smal", bufs=2))
                bmpp = p2.enter_context(tc.tile_pool(name="bmpp", bufs=1))
                pkp = p2.enter_context(tc.tile_pool(name="pkp", bufs=1))
                t0p = p2.enter_context(tc.tile_pool(name="t0p", bufs=2))
                sim_ps = p2.enter_context(
                    tc.tile_pool(name="sim_ps", bufs=1, space="PSUM")
                )
                combos = []
                for h in range(DT):
                    combos.append((fnt_myh[h], fnt_hi[h]))
                    combos.append((fnt_myh[h], fnt_lo[h]))
                    combos.append((fnt_myl[h], fnt_hi[h]))

                tkns = {}
                halves = {}

                def sim_block(m):
                    sh0 = simp.tile([128, N // 2], f32, name="sh0", tag="sh")
                    sh1 = simp.tile([128, N // 2], f32, name="sh1", tag="sh")
                    halves[m] = (sh0, sh1)
                    cand = smal.tile([128, 8 * NCH], f32, name="cand", tag="cand")
                    for qr in range(4):
                        pss = [
                            sim_ps.tile([128, 512], f32, name=f"sq{t}", tag=f"sq{t}")
                            for t in range(4)
                        ]
                        for ci, (la, ra) in enumerate(combos):
                            lt = la[:, m * 128 : (m + 1) * 128]
                            for t in range(4):
                                ntc = qr * 4 + t
                                nc.tensor.matmul(
                                    pss[t][:],
                                    lt,
                                    ra[:, ntc * 512 : (ntc + 1) * 512],
                                    start=(ci == 0),
                                    stop=(ci == 5),
                                )
                        sh = (sh0, sh1)[qr // 2]
                        for t in range(4):
                            ntc = qr * 4 + t
                            nc.vector.max(
                                cand[:, ntc * 8 : (ntc + 1) * 8], pss[t][:]
                            )
                            nc.scalar.copy(
                                sh[:, (ntc % 8) * 512 : (ntc % 8 + 1) * 512],
                                pss[t][:],
                            )
                    c8 = smal.tile([128, 8], f32, name="c8", tag="c8")
                    nc.vector.max(c8[:], cand[:])
                    cand2 = smal.tile([128, 8 * NCH], f32, name="cand2", tag="cand2")
                    nc.vector.match_replace(cand2[:], c8[:], cand[:], NEG)
                    c8b = smal.tile([128, 8], f32, name="c8b", tag="c8b")
                    nc.vector.max(c8b[:], cand2[:])
                    tkns[m] = c8b

                def mask_pack(m):
                    tkn = tkns[m][:, KN - 9 : KN - 8]
                    sh0, sh1 = halves[m]
                    bmp = bmpp.tile([128, BW], fp8, name="bmp")
                    for j in range(NCORES):
                        sh = (sh0, sh1)[j // 4]
                        nc.vector.tensor_scalar(
                            bmp[:, j * PAD : j * PAD + RP],
                            sh[:, (j % 4) * RP : (j % 4 + 1) * RP],
                            tkn,
                            None,
                            op0=Alu.is_ge,
                        )
                        nc.vector.memset(bmp[:, j * PAD + RP : (j + 1) * PAD], 0.0)
                    pk = pkp.tile([128, PCW], fp16, name="pk")
                    bm3 = bmp.rearrange("p (j t u) -> p j t u", j=NCORES, u=3)
                    for j in range(NCORES):
                        t0 = t0p.tile([128, PC], f32, name="t0")
                        nc.vector.scalar_tensor_tensor(
                            t0[:], in0=bm3[:, j, :, 1], scalar=BASE,
                            in1=bm3[:, j, :, 0], op0=Alu.mult, op1=Alu.add,
                        )
                        nc.vector.scalar_tensor_tensor(
                            pk[:, j * PC : (j + 1) * PC],
                            in0=bm3[:, j, :, 2], scalar=B2, in1=t0[:],
                            op0=Alu.mult, op1=Alu.add,
                        )
                    pp, i = m // 2, m % 2
                    nc.sync.dma_start(pk_in2[pp][i], pk[:])
                    nc.sync.dma_start(
                        b_grp2[pp][:, i].rearrange("j p q -> p j q"),
                        bmp.rearrange("p (j q) -> p j q", j=NCORES)[:, :, 0:RP],
                    )

                lt_dma = {}

                def cc_pair(pp):
                    nc.gpsimd.collective_compute(
                        "AllToAll", Alu.bypass, replica_groups=rg,
                        ins=[b_grp2[pp].opt()], outs=[lts_d2[pp].opt()],
                    )
                    nc.gpsimd.collective_compute(
                        "AllGather", Alu.bypass, replica_groups=rg,
                        ins=[pk_in2[pp].opt()], outs=[pk_ag2[pp].opt()],
                    )
                    for i in range(2):
                        mp = 2 * pp + i
                        lt_dma[mp] = nc.sync.dma_start(
                            lt_all[:, mp * NCORES : (mp + 1) * NCORES, :],
                            lts_d2[pp][:, i].rearrange("o p q -> p o q"),
                        )

                ltp = lt_all.rearrange("p (t i o) q -> p t i o q", i=2, o=NCORES)

                def dv_pair(pp):
                    first = True
                    for o in range(NCORES):
                        for m in range(MB):
                            mm = nc.tensor.matmul(
                                dv_ps[:, m : m + 1],
                                ltp[:, pp, :, o, m * 128 : (m + 1) * 128],
                                ones_dr[:, :, 0:1],
                                perf_mode=mybir.MatmulPerfMode.DoubleRow,
                                start=(pp == 0 and o == 0 and m == 0),
                                stop=(pp == NP - 1 and o == NCORES - 1 and m == MB - 1),
                                skip_group_check=True,
                            )
                            if first:
                                for mp in (2 * pp, 2 * pp + 1):
                                    _add_dep_helper(
                                        mm.ins, lt_dma[mp].ins, sync=True,
                                        reason="dv after lt load",
                                    )
                                first = False

                for m in range(MB):
                    sim_block(m)
                    if m >= 1:
                        mask_pack(m - 1)
                        if (m - 1) % 2 == 1:
                            cc_pair((m - 1) // 2)
                    if m >= 4 and m % 2 == 0:
                        dv_pair((m - 4) // 2)
                mask_pack(MB - 1)
                cc_pair(NP - 1)
                dv_pair(NP - 2)
                dv_pair(NP - 1)
                lt_dmas = lt_dma

        # ---------------- phase 3+4: dv scales, G blocks --------------------
        with ExitStack() as p4:
            gw = p4.enter_context(tc.tile_pool(name="gw", bufs=4))
            dv_sb = gw.tile([128, MB], f32, name="dv_sb")
            nc.vector.tensor_copy(dv_sb[:], dv_ps[:])
            dv_stack.close()
            csp = p4.enter_context(tc.tile_pool(name="csp", bufs=2))
            stg = p4.enter_context(tc.tile_pool(name="stg", bufs=2))
            esp = p4.enter_context(tc.tile_pool(name="esp", bufs=2))
            rhp = p4.enter_context(tc.tile_pool(name="rhp", bufs=MB + 1))
            g_ps = p4.enter_context(tc.tile_pool(name="g_ps", bufs=1, space="PSUM"))
            nc.scalar.dma_start(dv_my_d.rearrange("(m p) -> p m", p=128), dv_sb[:])
            nc.gpsimd.collective_compute(
                "AllGather", Alu.bypass, replica_groups=rg,
                ins=[dv_my_d.opt()], outs=[dv_full.opt()],
            )
            # rowscale: rs = invdv(my col block) * inv_de
            d1 = gw.tile([128, MB], f32, name="d1")
            nc.vector.tensor_scalar_max(d1[:], dv_sb[:], 1.0)
            sq = gw.tile([128, MB], f32, name="sqv")
            nc.scalar.sqrt(sq[:], d1[:])
            rc = gw.tile([128, MB], f32, name="rc")
            nc.vector.reciprocal(rc[:], sq[:])
            mk = gw.tile([128, MB], f32, name="mk")
            nc.vector.tensor_scalar(mk[:], dv_sb[:], 0.0, None, op0=Alu.is_gt)
            iv = gw.tile([128, MB], f32, name="iv")
            nc.vector.tensor_tensor(iv[:], rc[:], mk[:], op=Alu.mult)
            rs0 = gw.tile([128, MB], f32, name="rsu0")
            nc.vector.tensor_scalar_mul(rs0[:], iv[:], inv_de)

            # colscale source: cs = invdv over all N (from dv AllGather)
            q = N // 128
            dvw = gw.tile([128, q], f32, name="dvw")
            nc.scalar.dma_start(dvw[:], dv_full.rearrange("(cm p) -> p cm", p=128))
            d1w = gw.tile([128, q], f32, name="d1w")
            nc.vector.tensor_scalar_max(d1w[:], dvw[:], 1.0)
            sqw = gw.tile([128, q], f32, name="sqw")
            nc.scalar.sqrt(sqw[:], d1w[:])
            rcw = gw.tile([128, q], f32, name="rcw")
            nc.vector.reciprocal(rcw[:], sqw[:])
            mkw = gw.tile([128, q], f32, name="mkw")
            nc.vector.tensor_scalar(mkw[:], dvw[:], 0.0, None, op0=Alu.is_gt)
            ivw = gw.tile([128, q], f32, name="ivw")
            nc.vector.tensor_tensor(ivw[:], rcw[:], mkw[:], op=Alu.mult)
            nc.scalar.dma_start(cs_dram.rearrange("(cm p) -> p cm", p=128), ivw[:])
            cs2d = cs_dram.rearrange("(a n) -> a n", a=1)

            dgroups = [(d,) for d in range(ND)]
            for dg in dgroups:
                jds, csts, csjs = {}, {}, {}
                for d in dg:
                    jr = nc.sync.alloc_register(f"jr{d}")
                    nc.sync.reg_load(jr, jsel_sb[0:1, d : d + 1])
                    jds[d] = nc.sync.snap(
                        jr, donate=True, min_val=0, max_val=NCORES - 1
                    )
                    jra = nc.scalar.alloc_register(f"jra{d}")
                    nc.scalar.reg_load(jra, jsel_sb[0:1, d : d + 1])
                    jda = nc.scalar.snap(
                        jra, donate=True, min_val=0, max_val=NCORES - 1
                    )
                    cs_t = gw.tile([1, RP], f32, name="cs_t", tag=f"cs_t{d % 2}")
                    nc.scalar.dma_start(cs_t[:], cs2d[:, bass.ds(jda * RP, RP)])
                    csts[d] = cs_t
                rhss = {}
                for mg in range(2):
                    if mg == 0 or len(dg) > 1:
                        for mp in range(MB):
                            for d in dg:
                                rh = rhp.tile(
                                    [128, NCORES, PC], fp16, name="rh", tag="rh"
                                )
                                nc.sync.dma_start(
                                    rh[:],
                                    pk_ag2[mp // 2][:, mp % 2, :, :].rearrange(
                                        "o p c -> p o c"
                                    )[:, :, bass.ds(jds[d] * PC, PC)],
                                )
                                rhss[(d, mp)] = rh
                    pss = {}
                    for di, d in enumerate(dg):
                        for t in range(4):
                            pss[(d, t)] = g_ps.tile(
                                [128, 512], f32, name=f"gp{di}{t}", tag=f"gp{di * 4 + t}"
                            )
                    si = 0
                    for mp in range(MB):
                        for o in range(NCORES):
                            s = mp * NCORES + o
                            for d in dg:
                                for t in range(4):
                                    m = mg * 4 + t
                                    mm = nc.tensor.matmul(
                                        pss[(d, t)][:, 0:PC],
                                        lt_all[:, s, m * 128 : (m + 1) * 128],
                                        rhss[(d, mp)][:, o, :],
                                        start=(si == 0),
                                        stop=(si == KK - 1),
                                    )
                                    if o == 0 and t == 0 and d == dg[0]:
                                        _add_dep_helper(
                                            mm.ins, lt_dmas[mp].ins, sync=True,
                                            reason="G after lt load",
                                        )
                            si += 1
                    # evict to SBUF first (frees PSUM banks for csj + next mg)
                    ess = {}
                    for d in dg:
                        for t in range(4):
                            es = esp.tile([128, PC], f32, name="es", tag="es")
                            nc.scalar.copy(es[:], pss[(d, t)][:, 0:PC])
                            ess[(d, t)] = es
                    if mg == 0:
                        for di, d in enumerate(dg):
                            csj = csp.tile([128, PAD], f32, name="csj", tag=f"csj{d % 2}")
                            for hh in range(2):
                                cps = g_ps.tile(
                                    [128, 512], f32, name="cps", tag=f"gp{di * 4 + hh}"
                                )
                                nc.tensor.matmul(
                                    cps[:], ones1[:],
                                    csts[d][:, hh * 512 : (hh + 1) * 512],
                                    start=True, stop=True,
                                )
                                nc.scalar.copy(
                                    csj[:, hh * 512 : (hh + 1) * 512], cps[:]
                                )
                            nc.vector.memset(csj[:, RP:PAD], 0.0)
                            csjs[d] = csj
                    for d in dg:
                        csj3 = csjs[d].rearrange("p (t u) -> p t u", u=3)
                        for t in range(4):
                            m = mg * 4 + t
                            es = ess[(d, t)]
                            z2 = esp.tile([128, PC], f32, name="z2", tag="z2")
                            nc.vector.tensor_scalar(
                                z2[:], es[:], 1.0 / B2, CB2, op0=Alu.mult, op1=Alu.add
                            )
                            m2 = esp.tile([128, PC], f32, name="m2", tag="m2")
                            nc.vector.tensor_scalar(
                                m2[:], z2[:], MAGIC, MAGIC,
                                op0=Alu.add, op1=Alu.subtract,
                            )
                            r2 = esp.tile([128, PC], f32, name="r2", tag="r2")
                            nc.vector.scalar_tensor_tensor(
                                r2[:], in0=m2[:], scalar=-B2, in1=es[:],
                                op0=Alu.mult, op1=Alu.add,
                            )
                            z1 = esp.tile([128, PC], f32, name="z1", tag="z1")
                            nc.vector.tensor_scalar(
                                z1[:], r2[:], 1.0 / BASE, CB1,
                                op0=Alu.mult, op1=Alu.add,
                            )
                            m1 = esp.tile([128, PC], f32, name="m1", tag="m1")
                            nc.vector.tensor_scalar(
                                m1[:], z1[:], MAGIC, MAGIC,
                                op0=Alu.add, op1=Alu.subtract,
                            )
                            m0 = esp.tile([128, PC], f32, name="m0", tag="m0")
                            nc.vector.scalar_tensor_tensor(
                                m0[:], in0=m1[:], scalar=-BASE, in1=r2[:],
                                op0=Alu.mult, op1=Alu.add,
                            )
                            gs = stg.tile([128, PAD], f32, name="gs", tag="gs")
                            gs3 = gs.rearrange("p (t u) -> p t u", u=3)
                            for u, mu in ((2, m2), (1, m1), (0, m0)):
                                au = esp.tile([128, PC], f32, name=f"a{u}", tag="au")
                                nc.scalar.activation(
                                    au[:], mu[:], Act.Copy, scale=rs0[:, m : m + 1]
                                )
                                nc.vector.tensor_tensor(
                                    gs3[:, :, u], au[:], csj3[:, :, u], op=Alu.mult
                                )
                            nc.sync.dma_start(
                                g_out[m * 128 : (m + 1) * 128, d * RP : (d + 1) * RP],
                                gs[:, 0:RP],
                            )

    nc.compile()
    return nc


_CACHE = {}


def get_nc(N, D, KN, NCORES):
    key = (N, D, KN, NCORES)
    if key not in _CACHE:
        _CACHE[key] = build_nc(N, D, KN, NCORES)
    return _CACHE[key]


def kernel(feats, kn, _trace=False):
    feats = np.asarray(feats, dtype=np.float32)
    kn = int(kn)
    N, D = feats.shape
    NCORES = 8
    ND = 5
    RP = N // NCORES
    nc = get_nc(N, D, kn, NCORES)
    ident = np.eye(128, dtype=np.float32)
    in_maps = []
    for c in range(NCORES):
        jsel = np.zeros((1, 8), np.int32)
        for d in range(8):
            jsel[0, d] = (c + d) % NCORES
        in_maps.append({
            "feats_all": feats,
            "feats_my": feats[c * RP : (c + 1) * RP],
            "ident_in": ident,
            "jsel_in": jsel,
        })
    res = run_bass_kernel_spmd(
        nc, in_maps, core_ids=list(range(NCORES)), trace=_trace
    )
    out = np.empty((N, N), dtype=np.float32)
    for c in range(NCORES):
        g = res.results[c]["g_out"]  # [RP, ND*RP]
        for d in range(ND):
            j = (c + d) % NCORES
            if d == 4 and c >= 4:
                continue
            blk = g[:, d * RP : (d + 1) * RP]
            out[c * RP : (c + 1) * RP, j * RP : (j + 1) * RP] = blk
            if d != 0:
                out[j * RP : (j + 1) * RP, c * RP : (c + 1) * RP] = blk.T
    if _trace:
        return out, res
    return out


if __name__ == "__main__":
    inputs = {
        "feats": np.load("/tmp/feats.npy"),
        "kn": 10,
    }
    out = kernel(**inputs)
    print("out", out.shape, out.dtype, float(np.abs(out).max()))
